# revision 23
# baseline (speedup 1.0000x reference)
"""Trainium2 Bass kernel for C4AutoregressivePrintf (scatter_memory).

Data-parallel over 8 NeuronCores: each core handles 1024 rows of the
[8192, 4096] memory. The soft attend eq_gate(m, addr) weights are 1.0
at the addressed cell and ±2.1e-9 at distance 1-2 (f32), so
value = |memory[r, addr_r]| to within 4e-4 absolute — far inside the
2e-2 gate and empirically flipping no extra tokens. Design:

- Gather: 8 indirect DMAs (one per row-group; the SWDGE hardware
  consumes exactly one offset per partition per DMA), with flat
  indices r*4096+addr precomputed on the host from `addr`.
- Soft-gate args are built from window-local coordinates
  (u_p = x+0.5 - k_p*d), bit-identical to the reference's args in all
  transition regions (verified empirically). The p0 enumeration
  window is 12 (gates outside are exactly zero in f32).
- Digits: qt is exactly integer except in transition rows;
  digit = qt_int - 10*trunc(qt/10) in int32 reproduces the reference.
- Tokens: digits are stored high-to-low next to a newline column as
  int16, and ONE gpsimd local_scatter places the 7-element block
  [d_{n-1}..d_0, \\n] at columns n-6..n of each row's 16-wide slot
  (negative indices drop; spill lands in the previous slot's unused
  columns 10..15). The int16 canvas DMAs out contiguously and the
  host casts to f32.
- A dummy sigmoid after the consts DMA preloads the ACT table set so
  no table load lands on the critical path; count sigmoids and the
  big pair share the ACT engine; everything elementwise runs on DVE
  (Pool is busy generating gather descriptors).

Soft-gate arithmetic keeps the f32 sigmoid identity
(t+0.5)*sig(20t+10) - (t-0.5)*sig(20t-10), which saturates to exactly
1.0/0.0 on hardware.
"""

import os
import sys

for _p in ("/opt/trn_rl_repo", "/root/.axon_site/_ro/trn_rl_repo"):
    if _p not in sys.path:
        sys.path.insert(0, _p)

import numpy as np

import concourse.bacc as bacc
import concourse.bass as bass
import concourse.mybir as mybir
import concourse.tile as tile
from concourse.bass_utils import run_bass_kernel_spmd

F32 = mybir.dt.float32
I32 = mybir.dt.int32
I16 = mybir.dt.int16
AF = mybir.ActivationFunctionType
OP = mybir.AluOpType
AX = mybir.AxisListType

P = 128          # partitions
NCORES = 8
B_FULL = 8192
B = B_FULL // NCORES   # rows per core
C = B // P             # row groups per core (8)
M = 4096               # memory size
TOKW = 16              # token slot stride (7 live + 9 dump)

INV10 = float(np.float32(1.0) / np.float32(10.0))
INV100 = float(np.float32(1.0) / np.float32(100.0))

P345_QD = [0.0, 1000.0, 2000.0, 0.0, 10000.0, 0.0, 100000.0]
P345_D = [1000.0, 1000.0, 1000.0, 10000.0, 10000.0, 100000.0, 100000.0]
P345_QV = [0.0, 1.0, 2.0, 0.0, 1.0, 0.0, 1.0]
CNT_QD = [10.0, 100.0, 1000.0, 10000.0, 100000.0]

# gate-tile section layout (28 gate columns per row-group)
S0, S1, S2, S3 = 0, 12, 17, 21   # starts of p0|p1|p2|p345 sections
GW = 28
GT = C * GW                      # 224


def _tile_row(row) -> np.ndarray:
    """[w] -> [P, C*w] per-group tiled f32 constant."""
    return np.ascontiguousarray(
        np.broadcast_to(np.tile(np.asarray(row, np.float32), C), (P, C * len(row))))


def _int_row(row) -> np.ndarray:
    """[w] -> [P, C*w] per-group tiled int32 bit patterns in f32."""
    r = np.tile(np.asarray(row, np.int32), C)
    return np.ascontiguousarray(
        np.broadcast_to(r.view(np.float32), (P, C * len(row))))


def _build_consts() -> np.ndarray:
    i12 = np.arange(12, dtype=np.float32)
    i5 = np.arange(5, dtype=np.float32)
    i4 = np.arange(4, dtype=np.float32)
    # K16[c*7+k] = 16c + k - 6 (token scatter index base)
    k16 = (16 * (np.arange(C * 7) // 7) + np.arange(C * 7) % 7 - 6).astype(np.int32)
    parts = [
        _tile_row(i12),                                   # C12    [C*12]
        _tile_row(i12 + 1.0),                             # C12U
        _tile_row(i5 * 10.0),                             # C5     [C*5]
        _tile_row(i5 * 10.0 + 10.0),                      # C5U
        _tile_row(i4 * 100.0),                            # C4     [C*4]
        _tile_row(i4 * 100.0 + 100.0),                    # C4U
        _tile_row(P345_QD),                               # CL345  [C*7]
        _tile_row(np.array(P345_QD) + np.array(P345_D)),  # CU345  [C*7]
        _tile_row(CNT_QD),                                # CNT5   [C*5]
        _tile_row(i5),                                    # QV5    [C*5]
        _tile_row(i4),                                    # QV4    [C*4]
        _tile_row(P345_QV),                               # QV345  [C*7]
        np.broadcast_to(np.array([10.0, -10.0], np.float32), (P, 2)),  # BIAS
        _int_row([10]),                                   # TENI   [C]
        _int_row([48]),                                   # C48I   [C]
        _int_row([0]),                                    # IC0    [C]
        _int_row([2]),                                    # IC2    [C]
        _int_row([97]),                                   # IC97   [C]
        _int_row([8]),                                    # IC8    [C]
        _tile_row([10.0]),                                # F10    [C]
        _tile_row([100.0]),                               # F100   [C]
        np.ascontiguousarray(
            np.broadcast_to(k16.view(np.float32), (P, C * 7))),  # K16 [C*7]
    ]
    return np.ascontiguousarray(
        np.concatenate(parts, axis=1), dtype=np.float32).view(np.int32)


_W = [C * 12, C * 12, C * 5, C * 5, C * 4, C * 4, C * 7, C * 7, C * 5, C * 5,
      C * 4, C * 7, 2, C, C, C, C, C, C, C, C, C * 7]
_OFF = np.concatenate([[0], np.cumsum(_W)]).tolist()
(K_C12, K_C12U, K_C5, K_C5U, K_C4, K_C4U, K_CL345, K_CU345, K_CNT5, K_QV5,
 K_QV4, K_QV345, K_BIAS, K_TENI, K_C48I, K_IC0, K_IC2, K_IC97, K_IC8,
 K_F10, K_F100, K_K16, K_L) = _OFF

_CONSTS = _build_consts()
assert _CONSTS.shape == (P, K_L)

_NC = None


def _build_program():
    nc = bacc.Bacc(trn_type="TRN2", target_bir_lowering=False)

    mem_d = nc.declare_dram_parameter("memory", [B, M], F32, isOutput=False)
    idx_d = nc.declare_dram_parameter("idx", [B], I32, isOutput=False)
    cst_d = nc.declare_dram_parameter("consts", [P, K_L], I32, isOutput=False)
    tok_d = nc.declare_dram_parameter("tok", [B * TOKW], I16, isOutput=True)
    val_d = nc.declare_dram_parameter("val", [B], F32, isOutput=True)

    vec = nc.vector
    act = nc.scalar
    gps = nc.gpsimd

    mem_flat = mem_d[:].rearrange("a (b c) -> (a b) c", c=1)

    def t3(t, n):
        return t[:].rearrange("p (c w) -> p c w", w=n)

    with tile.TileContext(nc) as tc:
        with tc.tile_pool(name="pool", bufs=1) as pool:
            # ---- input DMAs: idx on SP queue (critical), consts on ACT ----
            idx = pool.tile([P, C], I32)
            nc.sync.dma_start(out=idx[:], in_=idx_d[:].rearrange("(p c) -> p c", p=P))
            cst = pool.tile([P, K_L], I32)
            act.dma_start(out=cst[:], in_=cst_d[:])

            def fsec(a, w):
                return cst[:, a:a + w].bitcast(F32)

            cC12 = fsec(K_C12, C * 12)
            cC12U = fsec(K_C12U, C * 12)
            cC5 = fsec(K_C5, C * 5)
            cC5U = fsec(K_C5U, C * 5)
            cC4 = fsec(K_C4, C * 4)
            cC4U = fsec(K_C4U, C * 4)
            cCL345 = fsec(K_CL345, C * 7)
            cCU345 = fsec(K_CU345, C * 7)
            cCNT5 = fsec(K_CNT5, C * 5)
            cQV5 = fsec(K_QV5, C * 5)
            cQV4 = fsec(K_QV4, C * 4)
            cQV345 = fsec(K_QV345, C * 7)
            bias_p = fsec(K_BIAS, 1)
            bias_m = fsec(K_BIAS + 1, 1)
            cTENI = cst[:, K_TENI:K_TENI + C]
            cC48I = cst[:, K_C48I:K_C48I + C]
            cIC0 = cst[:, K_IC0:K_IC0 + C]
            cIC2 = cst[:, K_IC2:K_IC2 + C]
            cIC97 = cst[:, K_IC97:K_IC97 + C]
            cIC8 = cst[:, K_IC8:K_IC8 + C]
            cF10 = fsec(K_F10, C)
            cF100 = fsec(K_F100, C)
            cK16 = cst[:, K_K16:K_K16 + C * 7]

            # preload the sigmoid ACT table set while DMAs are in flight
            dummy = pool.tile([P, 1], F32)
            act.activation(out=dummy[:], in_=bias_p, func=AF.Sigmoid,
                           scale=20.0, bias=bias_p)

            # ---- 8 per-group indirect gathers (one offset per partition) ----
            g = pool.tile([P, C], F32)
            for c in range(C):
                gps.indirect_dma_start(
                    out=g[:, c:c + 1], out_offset=None,
                    in_=mem_flat,
                    in_offset=bass.IndirectOffsetOnAxis(ap=idx[:, c:c + 1], axis=0),
                )

            # ---- attend value: x = |g| ----
            x = pool.tile([P, C], F32)
            vec.tensor_scalar(out=x[:].bitcast(I32), in0=g[:].bitcast(I32),
                              scalar1=0x7FFFFFFF, scalar2=None,
                              op0=OP.bitwise_and)
            nc.sync.dma_start(out=val_d[:].rearrange("(p c) -> p c", p=P), in_=x[:])

            t1 = pool.tile([P, C], F32)
            act.activation(out=t1[:], in_=x[:], func=AF.Copy, scale=INV10)
            t2 = pool.tile([P, C], F32)
            act.activation(out=t2[:], in_=x[:], func=AF.Copy, scale=INV100)

            xp = pool.tile([P, C], F32)
            vec.tensor_scalar(out=xp[:], in0=x[:], scalar1=0.5, scalar2=None,
                              op0=OP.add)

            # ---- count args + sigmoids ----
            argc = pool.tile([P, C * 5], F32)
            vec.tensor_tensor(out=t3(argc, 5), in0=xp[:].to_broadcast([P, C, 5]),
                              in1=t3(cCNT5, 5), op=OP.subtract)
            sac = pool.tile([P, C * 5], F32)
            act.activation(out=sac[:], in_=argc[:], func=AF.Sigmoid,
                           scale=20.0, bias=bias_p)
            sbc = pool.tile([P, C * 5], F32)
            act.activation(out=sbc[:], in_=argc[:], func=AF.Sigmoid,
                           scale=20.0, bias=bias_m)

            # ---- window bases (all on DVE; x-scaled copies from ACT) ----
            xi = pool.tile([P, C], I32)
            vec.tensor_copy(out=xi[:], in_=x[:])
            k0i = pool.tile([P, C], I32)
            vec.tensor_scalar(out=k0i[:], in0=xi[:], scalar1=5, scalar2=0,
                              op0=OP.subtract, op1=OP.max)
            vec.tensor_scalar(out=k0i[:], in0=k0i[:], scalar1=988, scalar2=None,
                              op0=OP.min)
            k0 = pool.tile([P, C], F32)
            vec.tensor_copy(out=k0[:], in_=k0i[:])
            u0 = pool.tile([P, C], F32)
            vec.tensor_tensor(out=u0[:], in0=xp[:], in1=k0[:], op=OP.subtract)

            v0 = pool.tile([P, C], F32)
            act.activation(out=v0[:], in_=u0[:], func=AF.Copy, scale=-1.0,
                           bias=1.0)

            t1i = pool.tile([P, C], I32)
            gps.tensor_copy(out=t1i[:], in_=t1[:])
            gps.tensor_tensor(out=t1i[:], in0=t1i[:], in1=cIC2, op=OP.subtract)
            gps.tensor_tensor(out=t1i[:], in0=t1i[:], in1=cIC0, op=OP.max)
            gps.tensor_tensor(out=t1i[:], in0=t1i[:], in1=cIC97, op=OP.min)
            k1 = pool.tile([P, C], F32)
            gps.tensor_copy(out=k1[:], in_=t1i[:])
            k1x = pool.tile([P, C], F32)
            gps.tensor_tensor(out=k1x[:], in0=k1[:], in1=cF10, op=OP.mult)
            u1 = pool.tile([P, C], F32)
            gps.tensor_tensor(out=u1[:], in0=xp[:], in1=k1x[:], op=OP.subtract)
            v1 = pool.tile([P, C], F32)
            act.activation(out=v1[:], in_=u1[:], func=AF.Copy, scale=-1.0,
                           bias=10.0)

            t2i = pool.tile([P, C], I32)
            gps.tensor_copy(out=t2i[:], in_=t2[:])
            gps.tensor_tensor(out=t2i[:], in0=t2i[:], in1=cIC2, op=OP.subtract)
            gps.tensor_tensor(out=t2i[:], in0=t2i[:], in1=cIC0, op=OP.max)
            gps.tensor_tensor(out=t2i[:], in0=t2i[:], in1=cIC8, op=OP.min)
            k2 = pool.tile([P, C], F32)
            gps.tensor_copy(out=k2[:], in_=t2i[:])
            k2x = pool.tile([P, C], F32)
            gps.tensor_tensor(out=k2x[:], in0=k2[:], in1=cF100, op=OP.mult)
            u2 = pool.tile([P, C], F32)
            gps.tensor_tensor(out=u2[:], in0=xp[:], in1=k2x[:], op=OP.subtract)
            v2 = pool.tile([P, C], F32)
            act.activation(out=v2[:], in_=u2[:], func=AF.Copy, scale=-1.0,
                           bias=100.0)

            # ---- arg tile [P, GT argl | GT argu] (all DVE) ----
            arg = pool.tile([P, 2 * GT], F32)
            argl = arg[:, :GT]
            argu = arg[:, GT:]

            def sec(t, lo, hi):
                return t.rearrange("p (c w) -> p c w", w=GW)[:, :, lo:hi]

            vec.tensor_tensor(out=sec(argl, S0, S1),
                              in0=u0[:].to_broadcast([P, C, 12]),
                              in1=t3(cC12, 12), op=OP.subtract)
            gps.tensor_tensor(out=sec(argu, S0, S1),
                              in0=v0[:].to_broadcast([P, C, 12]),
                              in1=t3(cC12, 12), op=OP.add)
            gps.tensor_tensor(out=sec(argl, S1, S2),
                              in0=u1[:].to_broadcast([P, C, 5]),
                              in1=t3(cC5, 5), op=OP.subtract)
            gps.tensor_tensor(out=sec(argu, S1, S2),
                              in0=v1[:].to_broadcast([P, C, 5]),
                              in1=t3(cC5, 5), op=OP.add)
            gps.tensor_tensor(out=sec(argl, S2, S3),
                              in0=u2[:].to_broadcast([P, C, 4]),
                              in1=t3(cC4, 4), op=OP.subtract)
            gps.tensor_tensor(out=sec(argu, S2, S3),
                              in0=v2[:].to_broadcast([P, C, 4]),
                              in1=t3(cC4, 4), op=OP.add)
            vec.tensor_tensor(out=sec(argl, S3, GW),
                              in0=xp[:].to_broadcast([P, C, 7]),
                              in1=t3(cCL345, 7), op=OP.subtract)
            gps.tensor_tensor(out=sec(argu, S3, GW),
                              in0=t3(cCU345, 7),
                              in1=xp[:].to_broadcast([P, C, 7]), op=OP.subtract)

            # ---- the two big sigmoids over [argl|argu] ----
            sa = pool.tile([P, 2 * GT], F32)
            act.activation(out=sa[:], in_=arg[:], func=AF.Sigmoid,
                           scale=20.0, bias=bias_p)
            sb = pool.tile([P, 2 * GT], F32)
            act.activation(out=sb[:], in_=arg[:], func=AF.Sigmoid,
                           scale=20.0, bias=bias_m)

            # ---- count tail + token scatter indices (ACT shadow, DVE) ----
            hc = pool.tile([P, C * 5], F32)
            vec.scalar_tensor_tensor(out=hc[:], in0=argc[:], scalar=0.5,
                                     in1=sac[:], op0=OP.add, op1=OP.mult)
            lc = pool.tile([P, C * 5], F32)
            vec.scalar_tensor_tensor(out=lc[:], in0=argc[:], scalar=0.5,
                                     in1=sbc[:], op0=OP.subtract, op1=OP.mult)
            vec.tensor_tensor(out=hc[:], in0=hc[:], in1=lc[:], op=OP.subtract)
            cs = pool.tile([P, C], F32)
            vec.tensor_reduce(out=cs[:], in_=t3(hc, 5), axis=AX.X, op=OP.add)
            cntf = pool.tile([P, C], F32)
            vec.tensor_scalar(out=cntf[:], in0=cs[:], scalar1=1.0, scalar2=None,
                              op0=OP.add)
            ni = pool.tile([P, C], I32)
            vec.tensor_copy(out=ni[:], in_=cntf[:])
            idxw = pool.tile([P, C * 7], I32)
            vec.tensor_tensor(out=t3(idxw, 7), in0=ni[:].to_broadcast([P, C, 7]),
                              in1=t3(cK16, 7), op=OP.add)
            idx16 = pool.tile([P, C * 7], I16)
            vec.tensor_copy(out=idx16[:], in_=idxw[:])

            # ---- qsel assembly (DVE, ACT shadow) ----
            qsel = pool.tile([P, GT], F32)
            vec.tensor_tensor(out=sec(qsel[:], S0, S1),
                              in0=xp[:].to_broadcast([P, C, 12]),
                              in1=sec(argl, S0, S1), op=OP.subtract)
            gps.tensor_tensor(out=sec(qsel[:], S1, S2),
                              in0=k1[:].to_broadcast([P, C, 5]),
                              in1=t3(cQV5, 5), op=OP.add)
            gps.tensor_tensor(out=sec(qsel[:], S2, S3),
                              in0=k2[:].to_broadcast([P, C, 4]),
                              in1=t3(cQV4, 4), op=OP.add)
            gps.tensor_copy(out=sec(qsel[:], S3, GW), in_=t3(cQV345, 7))

            # d7h newline column (int16 canvas source)
            d7h = pool.tile([P, C * 7], I16)
            vec.tensor_copy(out=t3(d7h, 7)[:, :, 6], in_=cTENI[:])

            # ---- soft thresholds -> gates, split DVE [0:SPL] / Pool [SPL:]
            SPL = 288
            ap05 = pool.tile([P, 2 * GT - SPL], F32)
            vec.tensor_scalar(out=ap05[:], in0=arg[:, SPL:], scalar1=0.5,
                              scalar2=None, op0=OP.add)
            am05 = pool.tile([P, 2 * GT - SPL], F32)
            vec.tensor_scalar(out=am05[:], in0=arg[:, SPL:], scalar1=0.5,
                              scalar2=None, op0=OP.subtract)
            hi = pool.tile([P, 2 * GT], F32)
            lo = pool.tile([P, 2 * GT], F32)
            vec.scalar_tensor_tensor(out=hi[:, :SPL], in0=arg[:, :SPL], scalar=0.5,
                                     in1=sa[:, :SPL], op0=OP.add, op1=OP.mult)
            gps.tensor_tensor(out=hi[:, SPL:], in0=ap05[:], in1=sa[:, SPL:],
                              op=OP.mult)
            vec.scalar_tensor_tensor(out=lo[:, :SPL], in0=arg[:, :SPL], scalar=0.5,
                                     in1=sb[:, :SPL], op0=OP.subtract, op1=OP.mult)
            gps.tensor_tensor(out=lo[:, SPL:], in0=am05[:], in1=sb[:, SPL:],
                              op=OP.mult)
            vec.tensor_tensor(out=hi[:, :SPL], in0=hi[:, :SPL], in1=lo[:, :SPL],
                              op=OP.subtract)
            gps.tensor_tensor(out=hi[:, SPL:], in0=hi[:, SPL:], in1=lo[:, SPL:],
                              op=OP.subtract)
            gate = pool.tile([P, GT], F32)
            vec.tensor_tensor(out=gate[:], in0=hi[:, :GT], in1=hi[:, GT:],
                              op=OP.mult)
            vec.tensor_tensor(out=gate[:], in0=gate[:], in1=qsel[:], op=OP.mult)

            # ---- segment reduces into reversed digit layout ----
            # (digit p lands in column 5-p so d7h[:, :, :6] reads high-to-low)
            qt = pool.tile([P, C * 6], F32)
            for p_i, (s0, s1) in enumerate([(S0, S1), (S1, S2), (S2, S3)]):
                vec.tensor_reduce(out=qt[:, (5 - p_i)::6],
                                  in_=sec(gate[:], s0, s1), axis=AX.X, op=OP.add)
            gcol = t3(gate[:], GW)
            # p3 = g21+g22+g23; p4 = g24+g25; p5 = g26+g27 (Pool strided adds)
            gps.tensor_tensor(out=qt[:, 2::6], in0=gcol[:, :, 21],
                              in1=gcol[:, :, 22], op=OP.add)
            gps.tensor_tensor(out=qt[:, 2::6], in0=qt[:, 2::6],
                              in1=gcol[:, :, 23], op=OP.add)
            gps.tensor_tensor(out=qt[:, 1::6], in0=gcol[:, :, 24],
                              in1=gcol[:, :, 25], op=OP.add)
            gps.tensor_tensor(out=qt[:, 0::6], in0=gcol[:, :, 26],
                              in1=gcol[:, :, 27], op=OP.add)

            # ---- digits -> d7h[:, :, :6] (int16, +48) ----
            qt6 = t3(qt, 6)
            d76 = t3(d7h, 7)
            # cols 0..2 (p5,p4,p3): qt < 10, mod-10 is identity (Pool)
            digP = pool.tile([P, C * 3], I32)
            gps.tensor_copy(out=t3(digP, 3), in_=qt6[:, :, 0:3])
            gps.tensor_tensor(out=t3(digP, 3), in0=t3(digP, 3),
                              in1=cC48I[:].to_broadcast([P, C, 3]), op=OP.add)
            gps.tensor_copy(out=d76[:, :, 0:3], in_=t3(digP, 3))
            # cols 3..5 (p2,p1,p0): digit = qti - 10*trunc(qt/10) (DVE, int)
            qti = pool.tile([P, C * 3], I32)
            vec.tensor_copy(out=t3(qti, 3), in_=qt6[:, :, 3:6])
            q10 = pool.tile([P, C * 3], F32)
            vec.tensor_scalar(out=t3(q10, 3), in0=qt6[:, :, 3:6],
                              scalar1=INV10, scalar2=None, op0=OP.mult)
            f10i = pool.tile([P, C * 3], I32)
            vec.tensor_copy(out=f10i[:], in_=q10[:])
            digD = pool.tile([P, C * 3], I32)
            vec.scalar_tensor_tensor(out=digD[:], in0=f10i[:], scalar=-10,
                                     in1=qti[:], op0=OP.mult, op1=OP.add)
            vec.tensor_scalar(out=digD[:], in0=digD[:], scalar1=48,
                              scalar2=None, op0=OP.add)
            vec.tensor_copy(out=d76[:, :, 3:6], in_=t3(digD, 3))

            # ---- token placement + output ----
            tok16 = pool.tile([P, C * TOKW], I16)
            gps.local_scatter(out_ap=tok16[:], data_ap=d7h[:], idxs_ap=idx16[:],
                              channels=P, num_elems=C * TOKW, num_idxs=C * 7)
            nc.sync.dma_start(
                out=tok_d[:].rearrange("(p c) -> p c", p=P), in_=tok16[:])
    nc.compile()
    return nc


def kernel(memory, addr, out_ptr):
    global _NC
    if _NC is None:
        _NC = _build_program()
    memory = np.ascontiguousarray(np.asarray(memory, dtype=np.float32))
    addr = np.asarray(addr, dtype=np.int64)
    rowbase = np.arange(B, dtype=np.int64) * M
    in_maps = []
    for c in range(NCORES):
        sl_ = slice(c * B, (c + 1) * B)
        in_maps.append({
            "memory": memory[sl_],
            "idx": (rowbase + addr[sl_]).astype(np.int32),
            "consts": _CONSTS,
        })
    res = run_bass_kernel_spmd(_NC, in_maps, list(range(NCORES)))
    out = np.zeros((B_FULL, 65), np.float32)
    for c, r in enumerate(res.results):
        sl_ = slice(c * B, (c + 1) * B)
        out[sl_, 0:7] = r["tok"].reshape(B, TOKW)[:, 0:7].astype(np.float32)
        out[sl_, 64] = r["val"]
    return out
